# revision 3
# baseline (speedup 1.0000x reference)
"""NeighborSample kernel for Trainium2 (8 NeuronCores, data-parallel over batch).

For input (8, 192, 64, 64) fp32, produces (8*64*64, 192, 5, 5): the 5x5
zero-padded neighborhood of every pixel, channels-major, neighborhood-minor.

Per-core strategy (1 batch image per core):
  - c-major image (192, 4096) lives in SBUF flat (row-wrapped, no width
    padding) with 192-pixel zero guard bands on both ends; a shifted
    neighborhood view is then a fully contiguous 128-pixel slice.
  - For each 128-pixel output tile and each of the 25 (i,j) shifts, TensorE
    transposes the shifted (96, 128) c-major slice into PSUM (128, 96)
    [pixel-major].  Out-of-image rows land in the guard zeros.
  - Vector/Scalar engines drain PSUM into an SBUF out tile (128, 4800) with
    stride-25/5 writes building the (c, 5, 5)-interleaved layout, multiplying
    by a per-partition mask that zeroes column-wrapped (x+j-2 out of range)
    lanes.
  - Contiguous 2.4 MB DMA per tile to HBM.
"""

import numpy as np

H = 64
W = 64
K = 5
PAD = 2
C = 192
CG = 96           # channels per partition group (2 groups)
NPIX = H * W
TP = 128          # pixels per tile (2 image rows)
NT = NPIX // TP   # 32 tiles
CK = C * K * K    # 4800
GUARD = 192       # zero guard pixels before/after the image
N_CORES = 8

_CACHE = {}


def _build_nc():
    import concourse.bacc as bacc
    import concourse.mybir as mybir
    from concourse.tile import TileContext

    fp32 = mybir.dt.float32
    nc = bacc.Bacc("TRN2", target_bir_lowering=False, debug=True)
    x = nc.dram_tensor("x", (C, NPIX), fp32, kind="ExternalInput")
    ident = nc.dram_tensor("ident", (CG, CG), fp32, kind="ExternalInput")
    jm = nc.dram_tensor("jm", (TP, K), fp32, kind="ExternalInput")
    y = nc.dram_tensor("y", (NPIX, CK), fp32, kind="ExternalOutput")

    BUFW = GUARD + NPIX + GUARD

    with TileContext(nc) as tc:
        with (
            tc.tile_pool(name="const", bufs=1) as constp,
            tc.tile_pool(name="bufs", bufs=1) as bufp,
            tc.tile_pool(name="psum", bufs=8, space="PSUM") as psump,
            tc.tile_pool(name="outp", bufs=4) as outp,
        ):
            ident_t = constp.tile([CG, CG], fp32, name="ident_t", tag="ident_t")
            nc.sync.dma_start(ident_t[:], ident[:])
            jm_t = constp.tile([TP, K], fp32, name="jm_t", tag="jm_t")
            nc.sync.dma_start(jm_t[:], jm[:])

            bufs = []
            for cg in range(2):
                b = bufp.tile([CG, BUFW], fp32, name=f"buf{cg}", tag=f"buf{cg}")
                nc.gpsimd.memset(b[:, 0:GUARD], 0.0)
                nc.gpsimd.memset(b[:, GUARD + NPIX:BUFW], 0.0)
                nc.sync.dma_start(
                    b[:, GUARD:GUARD + NPIX], x[cg * CG:(cg + 1) * CG, :]
                )
                bufs.append(b)

            # shift groups: fixed j, pairs of i -> same wrap mask per group
            groups = []
            for j in range(K):
                for i0 in (0, 2, 4):
                    ii = [i0] if i0 == 4 else [i0, i0 + 1]
                    groups.append((j, ii))

            op_idx = 0
            for t in range(NT):
                out_t = outp.tile([TP, CK], fp32, name="out_t", tag="out_t")
                out5 = out_t.rearrange(
                    "p (g c i j) -> p g c i j", g=2, c=CG, i=K, j=K
                )
                for (j, ii) in groups:
                    nil = len(ii)
                    psum_t = psump.tile([TP, 384], fp32, name="ps", tag="ps")
                    for il, i in enumerate(ii):
                        s = (i - PAD) * W + (j - PAD)
                        for cg in range(2):
                            src = bufs[cg][:, GUARD + TP * t + s:
                                           GUARD + TP * t + s + TP]
                            nc.tensor.transpose(
                                psum_t[:, il * 192 + cg * CG:
                                       il * 192 + (cg + 1) * CG],
                                src,
                                ident_t[:],
                            )
                    ps4 = psum_t.rearrange("p (il g c) -> p g c il", il=2, g=2, c=CG)
                    srcv = ps4[:, :, :, 0:nil]
                    dst = out5[:, :, :, ii[0]:ii[0] + nil, j]
                    mask = jm_t[:, j:j + 1]
                    if op_idx % 2 == 0:
                        nc.vector.tensor_scalar_mul(dst, srcv, mask)
                    else:
                        nc.scalar.mul(dst, srcv, mask)
                    op_idx += 1
                nc.sync.dma_start(y[t * TP:(t + 1) * TP, :], out_t[:])

    nc.finalize()
    return nc


def get_nc():
    if "nc" not in _CACHE:
        _CACHE["nc"] = _build_nc()
    return _CACHE["nc"]


def _make_jm() -> np.ndarray:
    jm = np.zeros((TP, K), dtype=np.float32)
    for p in range(TP):
        xcol = p % W
        for j in range(K):
            jm[p, j] = 1.0 if 0 <= xcol + j - PAD < W else 0.0
    return jm


def make_in_maps(inputs: np.ndarray):
    ident = np.eye(CG, dtype=np.float32)
    jm = _make_jm()
    return [
        {
            "x": np.ascontiguousarray(inputs[b].reshape(C, NPIX)),
            "ident": ident,
            "jm": jm,
        }
        for b in range(N_CORES)
    ]


def kernel(inputs: np.ndarray) -> np.ndarray:
    from concourse import bass_utils

    assert inputs.shape == (N_CORES, C, H, W)
    nc = get_nc()
    in_maps = make_in_maps(np.asarray(inputs, dtype=np.float32))
    res = bass_utils.run_bass_kernel_spmd(nc, in_maps, core_ids=list(range(N_CORES)))
    outs = [res.results[b]["y"].reshape(NPIX, C, K, K) for b in range(N_CORES)]
    return np.concatenate(outs, axis=0)
